# revision 8
# baseline (speedup 1.0000x reference)
"""Trainium2 Bass kernel for MinkowskiFeatureFusionBlock.

Computes, for N=1.5M points sharded across 8 NeuronCores:
    x = concat([backbone_F, text_feats[batch_idx]], 1) @ W
    out = relu(batchnorm_train(x) * gamma + beta)

Algorithm (avoids any gather and any transpose of the big tensor in pass 1):
  text contribution:  text_feats[idx] @ W[CB:] == (text_feats @ W[CB:])[idx]
                      == onehot(idx) @ T          with T = text_feats @ W[CB:]
  Pass 1 per shard:   G = [F | onehot(idx)].T @ [F | onehot(idx)] top rows:
                      G_FF = F.T F, G_FB = F.T M  (PE contracts over rows ->
                      natural [rows, ch] layout is already "lhsT")
                      counts = colsum of M.
  AllReduce(G, cnt) across 8 cores (45 KB -> ~11 us).
  BN stats from G:    sum_x   = W1.T colsum(F) + T.T cnt
                      sum_x^2 = diag(W1.T G_FF W1) + 2 diag(W1.T S.T T)
                                + (T*T).T cnt        (S.T = G_FB)
  Fold BN into new weights:  inv = gamma*rsqrt(var+eps)
                      W2[0:96]   = W1 * inv
                      W2[96:104] = T * inv + (beta - mean*inv)
  Pass 2 per tile:    out = relu([F | M] @ W2) via PE transpose of F and M
                      tiles, accumulate-free single matmul per 128-row tile.

Rows are padded per-shard with batch_idx=8 (one-hot all zero) so padding
contributes nothing to stats and produces relu(0)=0 outputs that are dropped.
"""

import numpy as np

import concourse.bacc as bacc
import concourse.mybir as mybir
import concourse.tile as tile
from concourse.bass_utils import run_bass_kernel_spmd

f32 = mybir.dt.float32
i32 = mybir.dt.int32

CB, CT, CO, B = 96, 128, 96, 8
KA = CB + B  # 104
NCORES = 8
P = 128
BN_EPS = 1e-5

N_TOTAL = 1_500_000
N_SHARD = N_TOTAL // NCORES  # 187500
J = 64                       # rows per partition per chunk
CHUNK_ROWS = P * J           # 8192
N_CHUNK = 23                 # ceil(187500 / 8192)
N_PAD = N_CHUNK * CHUNK_ROWS  # 188416
GRP = 8                      # row-tiles per psum group in pass 2

AluOp = mybir.AluOpType
ActFn = mybir.ActivationFunctionType


def build_nc(n_chunk=N_CHUNK, j_rows=J, n_total=N_TOTAL, n_cores=NCORES):
    n_pad = n_chunk * P * j_rows
    jr = j_rows
    nc = bacc.Bacc(None, target_bir_lowering=False, debug=False)

    F_ext = nc.dram_tensor("F", [n_pad, CB], f32, kind="ExternalInput")
    idx_ext = nc.dram_tensor("idx", [n_pad], i32, kind="ExternalInput")
    W_ext = nc.dram_tensor("W", [CB + CT, CO], f32, kind="ExternalInput")
    tf_ext = nc.dram_tensor("text", [B, CT], f32, kind="ExternalInput")
    gam_ext = nc.dram_tensor("gamma", [CO], f32, kind="ExternalInput")
    beta_ext = nc.dram_tensor("beta", [CO], f32, kind="ExternalInput")
    out_ext = nc.dram_tensor("out", [n_pad, CO], f32, kind="ExternalOutput")

    F_v = F_ext[:].rearrange("(ch p j) c -> ch p (j c)", p=P, j=jr)
    idx_v = idx_ext[:].rearrange("(ch p j) -> ch p j", p=P, j=jr)
    out_v = out_ext[:].rearrange("(ch p j) c -> ch p (j c)", p=P, j=jr)

    with tile.TileContext(nc) as tc:
        with (
            tc.tile_pool(name="const", bufs=1) as cpool,
            tc.tile_pool(name="io", bufs=3) as io,
            tc.tile_pool(name="mk", bufs=2) as mk,
            tc.tile_pool(name="dram", bufs=1, space="DRAM") as dram,
        ):
            # ---- constants ----
            iota_col = cpool.tile([P, 1], i32)
            nc.gpsimd.iota(iota_col[:], pattern=[[0, 1]], channel_multiplier=1)
            iota_row = cpool.tile([P, P], i32)
            nc.gpsimd.iota(iota_row[:], pattern=[[1, P]], channel_multiplier=0)
            iota_col_f = cpool.tile([P, 1], f32)
            nc.vector.tensor_copy(iota_col_f[:], iota_col[:])
            iota_row_f = cpool.tile([P, P], f32)
            nc.vector.tensor_copy(iota_row_f[:], iota_row[:])
            ident = cpool.tile([P, P], f32)
            nc.vector.tensor_scalar(
                ident[:], iota_row_f[:], iota_col_f[:], None, op0=AluOp.is_equal
            )
            ones_col = cpool.tile([P, 1], f32)
            nc.vector.memset(ones_col[:], 1.0)
            ones_row = cpool.tile([1, KA], f32)
            nc.vector.memset(ones_row[:], 1.0)
            cnt_acc = cpool.tile([P, B], f32)
            nc.vector.memset(cnt_acc[:], 0.0)

            def load_chunk_and_masks(ch):
                Fc = io.tile([P, jr * CB], f32, tag="fchunk")
                nc.sync.dma_start(Fc[:], F_v[ch])
                ic = io.tile([P, jr], i32, tag="idx")
                nc.sync.dma_start(ic[:], idx_v[ch])
                icf = mk.tile([P, jr], f32, tag="idxf")
                nc.vector.tensor_copy(icf[:], ic[:])
                M = mk.tile([P, B * jr], f32, tag="mask")
                for b in range(B):
                    nc.vector.tensor_scalar(
                        M[:, b * jr:(b + 1) * jr], icf[:], float(b), None,
                        op0=AluOp.is_equal,
                    )
                Mr = M[:].rearrange("p (b j) -> p b j", b=B)
                return Fc, Mr

            # ================= PASS 1: G = A.T A =================
            with tc.tile_pool(name="psg", bufs=1, space="PSUM") as psg:
                psum_G = psg.tile([CB, KA], f32)
                for ch in range(n_chunk):
                    Fc, Mr = load_chunk_and_masks(ch)
                    cntp = mk.tile([P, B], f32, tag="cntp")
                    nc.vector.tensor_reduce(
                        cntp[:], Mr, axis=mybir.AxisListType.X, op=AluOp.add
                    )
                    nc.vector.tensor_tensor(cnt_acc[:], cnt_acc[:], cntp[:], AluOp.add)
                    for jj in range(jr):
                        Fj = Fc[:, jj * CB:(jj + 1) * CB]
                        Bj = Mr[:, :, jj]
                        first = ch == 0 and jj == 0
                        last = ch == n_chunk - 1 and jj == jr - 1
                        # start=True clears the whole 2KB PSUM bank's
                        # has_written state, so only the very first matmul
                        # into this bank may set it; only the very last
                        # closes the group.
                        nc.tensor.matmul(
                            psum_G[:, 0:CB], Fj, Fj,
                            start=first, stop=False, skip_group_check=True,
                        )
                        nc.tensor.matmul(
                            psum_G[:, CB:KA], Fj, Bj,
                            start=False, stop=last, skip_group_check=True,
                        )
                Gl = cpool.tile([CB, KA], f32)
                nc.vector.tensor_copy(Gl[:], psum_G[:])

            # ============ AllReduce of [G | cnt] + BN stats ============
            with tc.tile_pool(name="pstat", bufs=4, space="PSUM") as ps:
                pcnt = ps.tile([B, 1], f32, tag="st")
                nc.tensor.matmul(pcnt[:], cnt_acc[:], ones_col[:], start=True, stop=True)
                cntl = cpool.tile([B, 1], f32)
                nc.vector.tensor_copy(cntl[:], pcnt[:])

                cc_in = dram.tile([CB + 1, KA], f32)
                cc_out = dram.tile([CB + 1, KA], f32)
                zeros_row = cpool.tile([1, KA], f32)
                nc.vector.memset(zeros_row[:], 0.0)
                nc.sync.dma_start(cc_in[0:CB, :], Gl[:])
                nc.sync.dma_start(cc_in[CB:CB + 1, 0:B], cntl[:])
                nc.sync.dma_start(cc_in[CB:CB + 1, B:KA], zeros_row[:, 0:KA - B])
                nc.gpsimd.collective_compute(
                    "AllReduce", AluOp.add,
                    replica_groups=[list(range(n_cores))],
                    ins=[cc_in.opt()], outs=[cc_out.opt()],
                )
                G_sb = cpool.tile([CB, KA], f32)
                nc.sync.dma_start(G_sb[:], cc_out[0:CB, :])
                cnt_sb = cpool.tile([B, 1], f32)
                nc.sync.dma_start(cnt_sb[:], cc_out[CB:CB + 1, 0:B])

                # ---- small-weights loads ----
                W1_sb = cpool.tile([CB, CO], f32)
                nc.sync.dma_start(W1_sb[:], W_ext[0:CB, :])
                Wt_sb = cpool.tile([CT, CO], f32)
                nc.sync.dma_start(Wt_sb[:], W_ext[CB:CB + CT, :])
                tf_sb = cpool.tile([B, CT], f32)
                nc.sync.dma_start(tf_sb[:], tf_ext[:])
                gam_sb = cpool.tile([CO, 1], f32)
                nc.sync.dma_start(gam_sb[:], gam_ext[:][:, None])
                beta_sb = cpool.tile([CO, 1], f32)
                nc.sync.dma_start(beta_sb[:], beta_ext[:][:, None])

                # T = text_feats @ W[CB:]  (via transpose of text_feats)
                p_tfT = ps.tile([CT, B], f32, tag="st")
                nc.tensor.transpose(p_tfT[:], tf_sb[:], ident[0:B, 0:B])
                tfT_sb = cpool.tile([CT, B], f32)
                nc.vector.tensor_copy(tfT_sb[:], p_tfT[:])
                p_T = ps.tile([B, CO], f32, tag="st")
                nc.tensor.matmul(p_T[:], tfT_sb[:], Wt_sb[:], start=True, stop=True)
                T_sb = cpool.tile([B, CO], f32)
                nc.vector.tensor_copy(T_sb[:], p_T[:])

                # S = (G_FB).T : [B, CB]
                p_S = ps.tile([B, CB], f32, tag="st")
                nc.tensor.transpose(p_S[:], G_sb[:, CB:KA], ident[0:CB, 0:CB])
                S_sb = cpool.tile([B, CB], f32)
                nc.vector.tensor_copy(S_sb[:], p_S[:])

                T2_sb = cpool.tile([B, CO], f32)
                nc.vector.tensor_scalar_mul(T2_sb[:], T_sb[:], 2.0)
                TT2_sb = cpool.tile([B, CO], f32)
                nc.vector.tensor_tensor(TT2_sb[:], T_sb[:], T_sb[:], AluOp.mult)

                # B1 = G_FF @ W1 + S.T @ (2T)
                p_B1 = ps.tile([CB, CO], f32, tag="st")
                nc.tensor.matmul(p_B1[:], G_sb[:, 0:CB], W1_sb[:], start=True, stop=False)
                nc.tensor.matmul(p_B1[:], S_sb[:], T2_sb[:], start=False, stop=True)
                Q_sb = cpool.tile([CB, CO], f32)
                nc.vector.tensor_tensor(Q_sb[:], W1_sb[:], p_B1[:], AluOp.mult)

                # E2 = colsum(Q) + (T*T).T @ cnt   [CO, 1]
                p_E2 = ps.tile([CO, 1], f32, tag="st")
                nc.tensor.matmul(p_E2[:], Q_sb[:], ones_col[0:CB, :], start=True, stop=False)
                nc.tensor.matmul(p_E2[:], TT2_sb[:], cnt_sb[:], start=False, stop=True)

                # mean = (W1.T colsumF + T.T cnt)/N
                colsF = cpool.tile([CB, 1], f32)
                nc.vector.tensor_reduce(
                    colsF[:], G_sb[:, CB:KA], axis=mybir.AxisListType.X, op=AluOp.add
                )
                p_mean = ps.tile([CO, 1], f32, tag="st")
                nc.tensor.matmul(p_mean[:], W1_sb[:], colsF[:], start=True, stop=False)
                nc.tensor.matmul(p_mean[:], T_sb[:], cnt_sb[:], start=False, stop=True)
                mean_sb = cpool.tile([CO, 1], f32)
                nc.vector.tensor_scalar_mul(mean_sb[:], p_mean[:], 1.0 / n_total)

                # var = E2/N - mean^2 ; inv = gamma / sqrt(var + eps)
                e2n = cpool.tile([CO, 1], f32)
                nc.vector.tensor_scalar_mul(e2n[:], p_E2[:], 1.0 / n_total)
                msq = cpool.tile([CO, 1], f32)
                nc.vector.tensor_tensor(msq[:], mean_sb[:], mean_sb[:], AluOp.mult)
                var_sb = cpool.tile([CO, 1], f32)
                nc.vector.tensor_tensor(var_sb[:], e2n[:], msq[:], AluOp.subtract)
                eps_sb = cpool.tile([CO, 1], f32)
                nc.vector.memset(eps_sb[:], BN_EPS)
                std_sb = cpool.tile([CO, 1], f32)
                nc.scalar.activation(std_sb[:], var_sb[:], ActFn.Sqrt, bias=eps_sb[:])
                rstd_sb = cpool.tile([CO, 1], f32)
                nc.vector.reciprocal(rstd_sb[:], std_sb[:])
                inv_sb = cpool.tile([CO, 1], f32)
                nc.vector.tensor_tensor(inv_sb[:], gam_sb[:], rstd_sb[:], AluOp.mult)
                mi_sb = cpool.tile([CO, 1], f32)
                nc.vector.tensor_tensor(mi_sb[:], mean_sb[:], inv_sb[:], AluOp.mult)
                bmi_sb = cpool.tile([CO, 1], f32)
                nc.vector.tensor_tensor(bmi_sb[:], beta_sb[:], mi_sb[:], AluOp.subtract)

                # rows: inv_row = inv.T, bmi_row = bmi.T  [1, CO]
                p_r1 = ps.tile([1, CO], f32, tag="st")
                nc.tensor.transpose(p_r1[:], inv_sb[:], ident[0:CO, 0:CO])
                inv_row = cpool.tile([1, CO], f32)
                nc.vector.tensor_copy(inv_row[:], p_r1[:])
                p_r2 = ps.tile([1, CO], f32, tag="st")
                nc.tensor.transpose(p_r2[:], bmi_sb[:], ident[0:CO, 0:CO])
                bmi_row = cpool.tile([1, CO], f32)
                nc.vector.tensor_copy(bmi_row[:], p_r2[:])

                # W2[0:CB] = W1 * inv (broadcast via rank-1 matmul)
                p_invb = ps.tile([KA, CO], f32, tag="st")
                nc.tensor.matmul(p_invb[:], ones_row[:], inv_row[:], start=True, stop=True)
                W2_sb = cpool.tile([KA, CO], f32)
                nc.vector.tensor_tensor(W2_sb[0:CB, :], W1_sb[:], p_invb[0:CB, :], AluOp.mult)
                # W2[CB:KA] = T * inv + (beta - mean*inv)
                p_w2b = ps.tile([B, CO], f32, tag="st")
                nc.tensor.matmul(p_w2b[:], ones_row[:, 0:B], bmi_row[:], start=True, stop=True)
                t8_sb = cpool.tile([B, CO], f32)
                nc.vector.tensor_tensor(t8_sb[:], T_sb[:], p_invb[0:B, :], AluOp.mult)
                t8b_sb = cpool.tile([B, CO], f32)
                nc.vector.tensor_tensor(t8b_sb[:], t8_sb[:], p_w2b[:], AluOp.add)
                nc.sync.dma_start(W2_sb[CB:KA, :], t8b_sb[:])

            # ================= PASS 2: out = relu(A @ W2) =================
            with (
                tc.tile_pool(name="p2t", bufs=2, space="PSUM") as p2t,
                tc.tile_pool(name="p2x", bufs=2, space="PSUM") as p2x,
            ):
                for ch in range(n_chunk):
                    Fc, Mr = load_chunk_and_masks(ch)
                    outc = io.tile([P, jr * CO], f32, tag="outchunk")
                    for g in range(jr // GRP):
                        pT = p2t.tile([KA, GRP * P], f32, tag="pT")
                        px = p2x.tile([P, GRP * P], f32, tag="px")
                        AT = io.tile([KA, GRP * P], f32, tag="at")
                        for k in range(GRP):
                            jj = g * GRP + k
                            nc.tensor.transpose(
                                pT[0:CB, k * P:(k + 1) * P],
                                Fc[:, jj * CB:(jj + 1) * CB], ident[:],
                            )
                            # B_j.T via a normal matmul (B_j.T @ I): walrus
                            # rejects transpose-mode outputs at PSUM
                            # partition offset != 0, but col-tiled normal
                            # matmuls may write at partition 96.
                            nc.tensor.matmul(
                                pT[CB:KA, k * P:(k + 1) * P], Mr[:, :, jj], ident[:],
                                start=True, stop=True, tile_position=(0, 96),
                            )
                        nc.vector.tensor_copy(AT[:], pT[:])
                        for k in range(GRP):
                            nc.tensor.matmul(
                                px[:, k * P:k * P + CO],
                                AT[:, k * P:(k + 1) * P], W2_sb[:],
                                start=True, stop=True,
                            )
                        px_view = px[:].rearrange("p (k c) -> p k c", c=P)[:, :, 0:CO]
                        o0 = g * GRP * CO
                        out_view = outc[:, o0:o0 + GRP * CO].rearrange(
                            "p (k c) -> p k c", c=CO
                        )
                        nc.scalar.activation(out_view, px_view, ActFn.Relu)
                    nc.scalar.dma_start(out_v[ch], outc[:])

    nc.compile()
    return nc


_NC_CACHE = {}


def _get_nc():
    key = (N_CHUNK, J, N_TOTAL, NCORES)
    if key not in _NC_CACHE:
        _NC_CACHE[key] = build_nc()
    return _NC_CACHE[key]


def _run(inputs, **spmd_kwargs):
    F = np.ascontiguousarray(np.asarray(inputs["backbone_F"], dtype=np.float32))
    idx = np.ascontiguousarray(np.asarray(inputs["batch_idx"], dtype=np.int32))
    W = np.ascontiguousarray(np.asarray(inputs["W"], dtype=np.float32))
    text = np.ascontiguousarray(np.asarray(inputs["text_feats"], dtype=np.float32))
    gamma = np.ascontiguousarray(np.asarray(inputs["gamma"], dtype=np.float32))
    beta = np.ascontiguousarray(np.asarray(inputs["beta"], dtype=np.float32))

    nc = _get_nc()
    in_maps = []
    for c in range(NCORES):
        Fs = np.zeros((N_PAD, CB), np.float32)
        Fs[:N_SHARD] = F[c * N_SHARD:(c + 1) * N_SHARD]
        ids = np.full((N_PAD,), B, np.int32)  # pad rows get out-of-range id
        ids[:N_SHARD] = idx[c * N_SHARD:(c + 1) * N_SHARD]
        in_maps.append(
            {"F": Fs, "idx": ids, "W": W, "text": text, "gamma": gamma, "beta": beta}
        )
    res = run_bass_kernel_spmd(nc, in_maps, list(range(NCORES)), **spmd_kwargs)
    out = np.concatenate(
        [res.results[c]["out"][:N_SHARD] for c in range(NCORES)], axis=0
    )
    return out, res


def kernel(**inputs) -> np.ndarray:
    return _run(inputs)[0]


# revision 14
# speedup vs baseline: 1.6073x; 1.6073x over previous
"""Trainium2 Bass kernel for MinkowskiFeatureFusionBlock.

Computes, for N=1.5M points sharded across 8 NeuronCores:
    x = concat([backbone_F, text_feats[batch_idx]], 1) @ W
    out = relu(batchnorm_train(x) * gamma + beta)

Algorithm (avoids any gather and any transpose of the big tensor in pass 1):
  text contribution:  text_feats[idx] @ W[CB:] == (text_feats @ W[CB:])[idx]
                      == onehot(idx) @ T          with T = text_feats @ W[CB:]
  Pass 1 per shard:   G = [F | onehot(idx)].T @ [F | onehot(idx)] top rows:
                      G_FF = F.T F, G_FB = F.T M  (PE contracts over rows ->
                      natural [rows, ch] layout is already "lhsT")
                      counts = colsum of M.
  AllReduce(G, cnt) across 8 cores (45 KB -> ~11 us).
  BN stats from G:    sum_x   = W1.T colsum(F) + T.T cnt
                      sum_x^2 = diag(W1.T G_FF W1) + 2 diag(W1.T S.T T)
                                + (T*T).T cnt        (S.T = G_FB)
  Fold BN into new weights:  inv = gamma*rsqrt(var+eps)
                      W2[0:96]   = W1 * inv
                      W2[96:104] = T * inv + (beta - mean*inv)
  Pass 2 per tile:    out = relu([F | M] @ W2) via PE transpose of F and M
                      tiles, accumulate-free single matmul per 128-row tile.

Rows are padded per-shard with batch_idx=8 (one-hot all zero) so padding
contributes nothing to stats and produces relu(0)=0 outputs that are dropped.
"""

import numpy as np

import concourse.bacc as bacc
import concourse.mybir as mybir
import concourse.tile as tile
from concourse.bass_utils import run_bass_kernel_spmd

f32 = mybir.dt.float32
bf16 = mybir.dt.bfloat16
i32 = mybir.dt.int32

CB, CT, CO, B = 96, 128, 96, 8
KA = CB + B  # 104
NCORES = 8
P = 128
BN_EPS = 1e-5

N_TOTAL = 1_500_000
N_SHARD = N_TOTAL // NCORES  # 187500
J = 64                       # rows per partition per chunk
CHUNK_ROWS = P * J           # 8192
N_CHUNK = 23                 # ceil(187500 / 8192)
N_PAD = N_CHUNK * CHUNK_ROWS  # 188416
GRP = 8                      # row-tiles per psum group in pass 2

AluOp = mybir.AluOpType
ActFn = mybir.ActivationFunctionType


def build_nc(n_chunk=N_CHUNK, j_rows=J, n_total=N_TOTAL, n_cores=NCORES):
    n_pad = n_chunk * P * j_rows
    jr = j_rows
    nc = bacc.Bacc(None, target_bir_lowering=False, debug=False)

    F_ext = nc.dram_tensor("F", [n_pad, CB], f32, kind="ExternalInput")
    idx_ext = nc.dram_tensor("idx", [n_pad], i32, kind="ExternalInput")
    W_ext = nc.dram_tensor("W", [CB + CT, CO], f32, kind="ExternalInput")
    tf_ext = nc.dram_tensor("text", [B, CT], f32, kind="ExternalInput")
    gam_ext = nc.dram_tensor("gamma", [CO], f32, kind="ExternalInput")
    beta_ext = nc.dram_tensor("beta", [CO], f32, kind="ExternalInput")
    out_ext = nc.dram_tensor("out", [n_pad, CO], f32, kind="ExternalOutput")

    F_v = F_ext[:].rearrange("(ch p j) c -> ch p (j c)", p=P, j=jr)
    idx_v = idx_ext[:].rearrange("(ch p j) -> ch p j", p=P, j=jr)
    out_v = out_ext[:].rearrange("(ch p j) c -> ch p (j c)", p=P, j=jr)

    with tile.TileContext(nc) as tc:
        with (
            tc.tile_pool(name="const", bufs=1) as cpool,
            tc.tile_pool(name="io", bufs=3) as io,
            tc.tile_pool(name="mk", bufs=2) as mk,
            tc.tile_pool(name="dram", bufs=1, space="DRAM") as dram,
        ):
            # ---- constants ----
            iota_col = cpool.tile([P, 1], i32)
            nc.gpsimd.iota(iota_col[:], pattern=[[0, 1]], channel_multiplier=1)
            iota_row = cpool.tile([P, P], i32)
            nc.gpsimd.iota(iota_row[:], pattern=[[1, P]], channel_multiplier=0)
            iota_col_f = cpool.tile([P, 1], f32)
            nc.vector.tensor_copy(iota_col_f[:], iota_col[:])
            iota_row_f = cpool.tile([P, P], f32)
            nc.vector.tensor_copy(iota_row_f[:], iota_row[:])
            ident = cpool.tile([P, P], f32)
            nc.vector.tensor_scalar(
                ident[:], iota_row_f[:], iota_col_f[:], None, op0=AluOp.is_equal
            )
            ident_bf = cpool.tile([P, P], bf16)
            nc.vector.tensor_copy(ident_bf[:], ident[:])
            ones_col = cpool.tile([P, 1], f32)
            nc.vector.memset(ones_col[:], 1.0)
            ones_row = cpool.tile([1, KA], f32)
            nc.vector.memset(ones_row[:], 1.0)
            cnt_acc = cpool.tile([P, B], f32)
            nc.vector.memset(cnt_acc[:], 0.0)

            def load_chunk_and_masks(ch):
                Fc = io.tile([P, jr * CB], f32, tag="fchunk")
                nc.sync.dma_start(Fc[:], F_v[ch])
                # bf16 copy for the PE path (fp32 matmuls run at 1/4 rate and
                # lower to 2 HW matmuls each). Cast on gpsimd, which is idle.
                Fb = io.tile([P, jr * CB], bf16, tag="fbf")
                nc.gpsimd.tensor_copy(Fb[:], Fc[:])
                ic = io.tile([P, jr], i32, tag="idx")
                nc.sync.dma_start(ic[:], idx_v[ch])
                icf = mk.tile([P, jr], f32, tag="idxf")
                nc.vector.tensor_copy(icf[:], ic[:])
                M = mk.tile([P, B * jr], bf16, tag="mask")
                for b in range(B):
                    nc.vector.tensor_scalar(
                        M[:, b * jr:(b + 1) * jr], icf[:], float(b), None,
                        op0=AluOp.is_equal,
                    )
                Mr = M[:].rearrange("p (b j) -> p b j", b=B)
                return Fb, Mr

            # ================= PASS 1: G = A.T A =================
            with tc.tile_pool(name="psg", bufs=1, space="PSUM") as psg:
                psum_G = psg.tile([CB, KA], f32)
                for ch in range(n_chunk):
                    Fc, Mr = load_chunk_and_masks(ch)
                    cntp = mk.tile([P, B], f32, tag="cntp")
                    nc.vector.tensor_reduce(
                        cntp[:], Mr, axis=mybir.AxisListType.X, op=AluOp.add
                    )
                    nc.vector.tensor_tensor(cnt_acc[:], cnt_acc[:], cntp[:], AluOp.add)
                    for jj in range(jr):
                        Fj = Fc[:, jj * CB:(jj + 1) * CB]
                        Bj = Mr[:, :, jj]
                        first = ch == 0 and jj == 0
                        last = ch == n_chunk - 1 and jj == jr - 1
                        # start=True clears the whole 2KB PSUM bank's
                        # has_written state, so only the very first matmul
                        # into this bank may set it; only the very last
                        # closes the group.
                        nc.tensor.matmul(
                            psum_G[:, 0:CB], Fj, Fj,
                            start=first, stop=False, skip_group_check=True,
                        )
                        nc.tensor.matmul(
                            psum_G[:, CB:KA], Fj, Bj,
                            start=False, stop=last, skip_group_check=True,
                        )
                Gl = cpool.tile([CB, KA], f32)
                nc.vector.tensor_copy(Gl[:], psum_G[:])

            # ============ AllReduce of [G | cnt] + BN stats ============
            with tc.tile_pool(name="pstat", bufs=4, space="PSUM") as ps:
                pcnt = ps.tile([B, 1], f32, tag="st")
                nc.tensor.matmul(pcnt[:], cnt_acc[:], ones_col[:], start=True, stop=True)
                cntl = cpool.tile([B, 1], f32)
                nc.vector.tensor_copy(cntl[:], pcnt[:])

                cc_in = dram.tile([CB + 1, KA], f32)
                cc_out = dram.tile([CB + 1, KA], f32)
                zeros_row = cpool.tile([1, KA], f32)
                nc.vector.memset(zeros_row[:], 0.0)
                nc.sync.dma_start(cc_in[0:CB, :], Gl[:])
                nc.sync.dma_start(cc_in[CB:CB + 1, 0:B], cntl[:])
                nc.sync.dma_start(cc_in[CB:CB + 1, B:KA], zeros_row[:, 0:KA - B])
                nc.gpsimd.collective_compute(
                    "AllReduce", AluOp.add,
                    replica_groups=[list(range(n_cores))],
                    ins=[cc_in.opt()], outs=[cc_out.opt()],
                )
                G_sb = cpool.tile([CB, KA], f32)
                nc.sync.dma_start(G_sb[:], cc_out[0:CB, :])
                cnt_sb = cpool.tile([B, 1], f32)
                nc.sync.dma_start(cnt_sb[:], cc_out[CB:CB + 1, 0:B])

                # ---- small-weights loads ----
                W1_sb = cpool.tile([CB, CO], f32)
                nc.sync.dma_start(W1_sb[:], W_ext[0:CB, :])
                Wt_sb = cpool.tile([CT, CO], f32)
                nc.sync.dma_start(Wt_sb[:], W_ext[CB:CB + CT, :])
                tf_sb = cpool.tile([B, CT], f32)
                nc.sync.dma_start(tf_sb[:], tf_ext[:])
                gam_sb = cpool.tile([CO, 1], f32)
                nc.sync.dma_start(gam_sb[:], gam_ext[:][:, None])
                beta_sb = cpool.tile([CO, 1], f32)
                nc.sync.dma_start(beta_sb[:], beta_ext[:][:, None])

                # T = text_feats @ W[CB:]  (via transpose of text_feats)
                p_tfT = ps.tile([CT, B], f32, tag="st")
                nc.tensor.transpose(p_tfT[:], tf_sb[:], ident[0:B, 0:B])
                tfT_sb = cpool.tile([CT, B], f32)
                nc.vector.tensor_copy(tfT_sb[:], p_tfT[:])
                p_T = ps.tile([B, CO], f32, tag="st")
                nc.tensor.matmul(p_T[:], tfT_sb[:], Wt_sb[:], start=True, stop=True)
                T_sb = cpool.tile([B, CO], f32)
                nc.vector.tensor_copy(T_sb[:], p_T[:])

                # S = (G_FB).T : [B, CB]
                p_S = ps.tile([B, CB], f32, tag="st")
                nc.tensor.transpose(p_S[:], G_sb[:, CB:KA], ident[0:CB, 0:CB])
                S_sb = cpool.tile([B, CB], f32)
                nc.vector.tensor_copy(S_sb[:], p_S[:])

                T2_sb = cpool.tile([B, CO], f32)
                nc.vector.tensor_scalar_mul(T2_sb[:], T_sb[:], 2.0)
                TT2_sb = cpool.tile([B, CO], f32)
                nc.vector.tensor_tensor(TT2_sb[:], T_sb[:], T_sb[:], AluOp.mult)

                # B1 = G_FF @ W1 + S.T @ (2T)
                p_B1 = ps.tile([CB, CO], f32, tag="st")
                nc.tensor.matmul(p_B1[:], G_sb[:, 0:CB], W1_sb[:], start=True, stop=False)
                nc.tensor.matmul(p_B1[:], S_sb[:], T2_sb[:], start=False, stop=True)
                Q_sb = cpool.tile([CB, CO], f32)
                nc.vector.tensor_tensor(Q_sb[:], W1_sb[:], p_B1[:], AluOp.mult)

                # E2 = colsum(Q) + (T*T).T @ cnt   [CO, 1]
                p_E2 = ps.tile([CO, 1], f32, tag="st")
                nc.tensor.matmul(p_E2[:], Q_sb[:], ones_col[0:CB, :], start=True, stop=False)
                nc.tensor.matmul(p_E2[:], TT2_sb[:], cnt_sb[:], start=False, stop=True)

                # mean = (W1.T colsumF + T.T cnt)/N
                colsF = cpool.tile([CB, 1], f32)
                nc.vector.tensor_reduce(
                    colsF[:], G_sb[:, CB:KA], axis=mybir.AxisListType.X, op=AluOp.add
                )
                p_mean = ps.tile([CO, 1], f32, tag="st")
                nc.tensor.matmul(p_mean[:], W1_sb[:], colsF[:], start=True, stop=False)
                nc.tensor.matmul(p_mean[:], T_sb[:], cnt_sb[:], start=False, stop=True)
                mean_sb = cpool.tile([CO, 1], f32)
                nc.vector.tensor_scalar_mul(mean_sb[:], p_mean[:], 1.0 / n_total)

                # var = E2/N - mean^2 ; inv = gamma / sqrt(var + eps)
                e2n = cpool.tile([CO, 1], f32)
                nc.vector.tensor_scalar_mul(e2n[:], p_E2[:], 1.0 / n_total)
                msq = cpool.tile([CO, 1], f32)
                nc.vector.tensor_tensor(msq[:], mean_sb[:], mean_sb[:], AluOp.mult)
                var_sb = cpool.tile([CO, 1], f32)
                nc.vector.tensor_tensor(var_sb[:], e2n[:], msq[:], AluOp.subtract)
                eps_sb = cpool.tile([CO, 1], f32)
                nc.vector.memset(eps_sb[:], BN_EPS)
                std_sb = cpool.tile([CO, 1], f32)
                nc.scalar.activation(std_sb[:], var_sb[:], ActFn.Sqrt, bias=eps_sb[:])
                rstd_sb = cpool.tile([CO, 1], f32)
                nc.vector.reciprocal(rstd_sb[:], std_sb[:])
                inv_sb = cpool.tile([CO, 1], f32)
                nc.vector.tensor_tensor(inv_sb[:], gam_sb[:], rstd_sb[:], AluOp.mult)
                mi_sb = cpool.tile([CO, 1], f32)
                nc.vector.tensor_tensor(mi_sb[:], mean_sb[:], inv_sb[:], AluOp.mult)
                bmi_sb = cpool.tile([CO, 1], f32)
                nc.vector.tensor_tensor(bmi_sb[:], beta_sb[:], mi_sb[:], AluOp.subtract)

                # rows: inv_row = inv.T, bmi_row = bmi.T  [1, CO]
                p_r1 = ps.tile([1, CO], f32, tag="st")
                nc.tensor.transpose(p_r1[:], inv_sb[:], ident[0:CO, 0:CO])
                inv_row = cpool.tile([1, CO], f32)
                nc.vector.tensor_copy(inv_row[:], p_r1[:])
                p_r2 = ps.tile([1, CO], f32, tag="st")
                nc.tensor.transpose(p_r2[:], bmi_sb[:], ident[0:CO, 0:CO])
                bmi_row = cpool.tile([1, CO], f32)
                nc.vector.tensor_copy(bmi_row[:], p_r2[:])

                # W2[0:CB] = W1 * inv (broadcast via rank-1 matmul)
                p_invb = ps.tile([KA, CO], f32, tag="st")
                nc.tensor.matmul(p_invb[:], ones_row[:], inv_row[:], start=True, stop=True)
                W2_sb = cpool.tile([KA, CO], f32)
                nc.vector.tensor_tensor(W2_sb[0:CB, :], W1_sb[:], p_invb[0:CB, :], AluOp.mult)
                # W2[CB:KA] = T * inv + (beta - mean*inv)
                p_w2b = ps.tile([B, CO], f32, tag="st")
                nc.tensor.matmul(p_w2b[:], ones_row[:, 0:B], bmi_row[:], start=True, stop=True)
                t8_sb = cpool.tile([B, CO], f32)
                nc.vector.tensor_tensor(t8_sb[:], T_sb[:], p_invb[0:B, :], AluOp.mult)
                t8b_sb = cpool.tile([B, CO], f32)
                nc.vector.tensor_tensor(t8b_sb[:], t8_sb[:], p_w2b[:], AluOp.add)
                nc.sync.dma_start(W2_sb[CB:KA, :], t8b_sb[:])
                W2_bf = cpool.tile([KA, CO], bf16)
                nc.vector.tensor_copy(W2_bf[:], W2_sb[:])

            # ================= PASS 2: out = relu(A @ W2) =================
            with (
                tc.tile_pool(name="p2t", bufs=2, space="PSUM") as p2t,
                tc.tile_pool(name="p2x", bufs=2, space="PSUM") as p2x,
            ):
                for ch in range(n_chunk):
                    Fb, Mr = load_chunk_and_masks(ch)
                    outc = io.tile([P, jr * CO], f32, tag="outchunk")
                    for g in range(jr // GRP):
                        pT = p2t.tile([KA, GRP * P], f32, tag="pT")
                        px = p2x.tile([P, GRP * P], f32, tag="px")
                        AT = io.tile([KA, GRP * P], bf16, tag="at")
                        for k in range(GRP):
                            jj = g * GRP + k
                            # Transposes as normal matmuls against the
                            # identity: bf16 inputs stream at full rate into
                            # fp32 PSUM (transpose-mode would force a bf16
                            # PSUM tile and rejects partition offsets).
                            nc.tensor.matmul(
                                pT[0:CB, k * P:(k + 1) * P],
                                Fb[:, jj * CB:(jj + 1) * CB], ident_bf[:],
                                start=True, stop=True,
                            )
                            nc.tensor.matmul(
                                pT[CB:KA, k * P:(k + 1) * P], Mr[:, :, jj],
                                ident_bf[:],
                                start=True, stop=True, tile_position=(0, 96),
                            )
                        # PSUM->SBUF copy split across DVE and ACT for balance
                        nc.vector.tensor_copy(
                            AT[:, 0:GRP * P // 2], pT[:, 0:GRP * P // 2]
                        )
                        nc.scalar.copy(
                            AT[:, GRP * P // 2:], pT[:, GRP * P // 2:]
                        )
                        for k in range(GRP):
                            nc.tensor.matmul(
                                px[:, k * P:k * P + CO],
                                AT[:, k * P:(k + 1) * P], W2_bf[:],
                                start=True, stop=True,
                            )
                        px_view = px[:].rearrange("p (k c) -> p k c", c=P)[:, :, 0:CO]
                        o0 = g * GRP * CO
                        out_view = outc[:, o0:o0 + GRP * CO].rearrange(
                            "p (k c) -> p k c", c=CO
                        )
                        nc.scalar.activation(out_view, px_view, ActFn.Relu)
                    nc.scalar.dma_start(out_v[ch], outc[:])

    nc.compile()
    return nc


_NC_CACHE = {}


def _get_nc():
    key = (N_CHUNK, J, N_TOTAL, NCORES)
    if key not in _NC_CACHE:
        _NC_CACHE[key] = build_nc()
    return _NC_CACHE[key]


def _run(inputs, **spmd_kwargs):
    F = np.ascontiguousarray(np.asarray(inputs["backbone_F"], dtype=np.float32))
    idx = np.ascontiguousarray(np.asarray(inputs["batch_idx"], dtype=np.int32))
    W = np.ascontiguousarray(np.asarray(inputs["W"], dtype=np.float32))
    text = np.ascontiguousarray(np.asarray(inputs["text_feats"], dtype=np.float32))
    gamma = np.ascontiguousarray(np.asarray(inputs["gamma"], dtype=np.float32))
    beta = np.ascontiguousarray(np.asarray(inputs["beta"], dtype=np.float32))

    nc = _get_nc()
    in_maps = []
    for c in range(NCORES):
        Fs = np.zeros((N_PAD, CB), np.float32)
        Fs[:N_SHARD] = F[c * N_SHARD:(c + 1) * N_SHARD]
        ids = np.full((N_PAD,), B, np.int32)  # pad rows get out-of-range id
        ids[:N_SHARD] = idx[c * N_SHARD:(c + 1) * N_SHARD]
        in_maps.append(
            {"F": Fs, "idx": ids, "W": W, "text": text, "gamma": gamma, "beta": beta}
        )
    res = run_bass_kernel_spmd(nc, in_maps, list(range(NCORES)), **spmd_kwargs)
    out = np.concatenate(
        [res.results[c]["out"][:N_SHARD] for c in range(NCORES)], axis=0
    )
    return out, res


def kernel(**inputs) -> np.ndarray:
    return _run(inputs)[0]


# revision 19
# speedup vs baseline: 2.5144x; 1.5643x over previous
"""Trainium2 Bass kernel for MinkowskiFeatureFusionBlock.

Computes, for N=1.5M points sharded across 8 NeuronCores:
    x = concat([backbone_F, text_feats[batch_idx]], 1) @ W
    out = relu(batchnorm_train(x) * gamma + beta)

Algorithm (avoids any gather and any transpose of the big tensor in pass 1):
  text contribution:  text_feats[idx] @ W[CB:] == (text_feats @ W[CB:])[idx]
                      == onehot(idx) @ T          with T = text_feats @ W[CB:]
  Pass 1 per shard:   G = [F | onehot(idx)].T @ [F | onehot(idx)] top rows:
                      G_FF = F.T F, G_FB = F.T M  (PE contracts over rows ->
                      natural [rows, ch] layout is already "lhsT")
                      counts = colsum of M.
  AllReduce(G, cnt) across 8 cores (45 KB -> ~11 us).
  BN stats from G:    sum_x   = W1.T colsum(F) + T.T cnt
                      sum_x^2 = diag(W1.T G_FF W1) + 2 diag(W1.T S.T T)
                                + (T*T).T cnt        (S.T = G_FB)
  Fold BN into new weights:  inv = gamma*rsqrt(var+eps)
                      W2[0:96]   = W1 * inv
                      W2[96:104] = T * inv + (beta - mean*inv)
  Pass 2 per tile:    out = relu([F | M] @ W2) via PE transpose of F and M
                      tiles, accumulate-free single matmul per 128-row tile.

Rows are padded per-shard with batch_idx=8 (one-hot all zero) so padding
contributes nothing to stats and produces relu(0)=0 outputs that are dropped.
"""

import numpy as np

import concourse.bacc as bacc
import concourse.mybir as mybir
import concourse.tile as tile
from concourse.bass_utils import run_bass_kernel_spmd

f32 = mybir.dt.float32
bf16 = mybir.dt.bfloat16
i32 = mybir.dt.int32

CB, CT, CO, B = 96, 128, 96, 8
KA = CB + B  # 104
NCORES = 8
P = 128
BN_EPS = 1e-5

N_TOTAL = 1_500_000
N_SHARD = N_TOTAL // NCORES  # 187500
J = 64                       # rows per partition per chunk
CHUNK_ROWS = P * J           # 8192
N_CHUNK = 23                 # ceil(187500 / 8192)
N_PAD = N_CHUNK * CHUNK_ROWS  # 188416
GRP = 8                      # row-tiles per psum group in pass 2

AluOp = mybir.AluOpType
ActFn = mybir.ActivationFunctionType


def build_nc(n_chunk=N_CHUNK, j_rows=J, n_total=N_TOTAL, n_cores=NCORES):
    n_pad = n_chunk * P * j_rows
    jr = j_rows
    nc = bacc.Bacc(None, target_bir_lowering=False, debug=False)

    F_ext = nc.dram_tensor("F", [n_pad, CB], f32, kind="ExternalInput")
    idx_ext = nc.dram_tensor("idx", [n_pad], i32, kind="ExternalInput")
    W_ext = nc.dram_tensor("W", [CB + CT, CO], f32, kind="ExternalInput")
    tf_ext = nc.dram_tensor("text", [B, CT], f32, kind="ExternalInput")
    gam_ext = nc.dram_tensor("gamma", [CO], f32, kind="ExternalInput")
    beta_ext = nc.dram_tensor("beta", [CO], f32, kind="ExternalInput")
    out_ext = nc.dram_tensor("out", [n_pad, CO], f32, kind="ExternalOutput")

    F_v = F_ext[:].rearrange("(ch p j) c -> ch p (j c)", p=P, j=jr)
    idx_v = idx_ext[:].rearrange("(ch p j) -> ch p j", p=P, j=jr)
    out_v = out_ext[:].rearrange("(ch p j) c -> ch p (j c)", p=P, j=jr)

    with tile.TileContext(nc) as tc:
        with (
            tc.tile_pool(name="const", bufs=1) as cpool,
            tc.tile_pool(name="fb", bufs=4) as fbpool,
            tc.tile_pool(name="io", bufs=2) as io,
            tc.tile_pool(name="at", bufs=3) as atpool,
            tc.tile_pool(name="mk", bufs=2) as mk,
            tc.tile_pool(name="dram", bufs=1, space="DRAM") as dram,
        ):
            # ---- constants ----
            iota_col = cpool.tile([P, 1], i32)
            nc.gpsimd.iota(iota_col[:], pattern=[[0, 1]], channel_multiplier=1)
            iota_row = cpool.tile([P, P], i32)
            nc.gpsimd.iota(iota_row[:], pattern=[[1, P]], channel_multiplier=0)
            iota_col_f = cpool.tile([P, 1], f32)
            nc.vector.tensor_copy(iota_col_f[:], iota_col[:])
            iota_row_f = cpool.tile([P, P], f32)
            nc.vector.tensor_copy(iota_row_f[:], iota_row[:])
            ident = cpool.tile([P, P], f32)
            nc.vector.tensor_scalar(
                ident[:], iota_row_f[:], iota_col_f[:], None, op0=AluOp.is_equal
            )
            ident_bf = cpool.tile([P, P], bf16)
            nc.vector.tensor_copy(ident_bf[:], ident[:])
            ones_col = cpool.tile([P, 1], f32)
            nc.vector.memset(ones_col[:], 1.0)
            ones_row = cpool.tile([1, KA], f32)
            nc.vector.memset(ones_row[:], 1.0)
            cnt_acc = cpool.tile([P, B], f32)
            nc.vector.memset(cnt_acc[:], 0.0)

            # per-batch one-hot masks for every chunk, built once in pass 1
            # and persisted in bf16 for pass 2 (23 KB/partition).
            M_all = cpool.tile([P, n_chunk * B * jr], bf16)

            def load_fb(ch):
                # Load F directly as bf16: SWDGE casts fp32->bf16 in the DMA
                # datapath (fp32 matmuls run at 1/4 PE rate and lower to two
                # HW matmuls each, so the whole PE path is bf16).
                Fb = fbpool.tile([P, jr * CB], bf16, tag="fbf")
                nc.gpsimd.dma_start(Fb[:], F_v[ch])
                return Fb

            def mask_view(ch):
                return M_all[:, ch * B * jr:(ch + 1) * B * jr].rearrange(
                    "p (b j) -> p b j", b=B
                )

            # ================= PASS 1: G = A.T A =================
            with tc.tile_pool(name="psg", bufs=1, space="PSUM") as psg:
                psum_G = psg.tile([CB, KA], f32)
                for ch in range(n_chunk):
                    Fb = load_fb(ch)
                    ic = mk.tile([P, jr], i32, tag="idx")
                    nc.sync.dma_start(ic[:], idx_v[ch])
                    icf = mk.tile([P, jr], f32, tag="idxf")
                    nc.vector.tensor_copy(icf[:], ic[:])
                    Mf = mk.tile([P, B * jr], f32, tag="mask")
                    for b in range(B):
                        nc.vector.tensor_scalar(
                            Mf[:, b * jr:(b + 1) * jr], icf[:], float(b), None,
                            op0=AluOp.is_equal,
                        )
                    nc.vector.tensor_copy(
                        M_all[:, ch * B * jr:(ch + 1) * B * jr], Mf[:]
                    )
                    Mr = mask_view(ch)
                    Mfr = Mf[:].rearrange("p (b j) -> p b j", b=B)
                    cntp = mk.tile([P, B], f32, tag="cntp")
                    nc.vector.tensor_reduce(
                        cntp[:], Mfr, axis=mybir.AxisListType.X, op=AluOp.add
                    )
                    nc.vector.tensor_tensor(cnt_acc[:], cnt_acc[:], cntp[:], AluOp.add)
                    for jj in range(jr):
                        Fj = Fb[:, jj * CB:(jj + 1) * CB]
                        Bj = Mr[:, :, jj]
                        first = ch == 0 and jj == 0
                        last = ch == n_chunk - 1 and jj == jr - 1
                        # start=True clears the whole 2KB PSUM bank's
                        # has_written state, so only the very first matmul
                        # into this bank may set it; only the very last
                        # closes the group.
                        nc.tensor.matmul(
                            psum_G[:, 0:CB], Fj, Fj,
                            start=first, stop=False, skip_group_check=True,
                        )
                        nc.tensor.matmul(
                            psum_G[:, CB:KA], Fj, Bj,
                            start=False, stop=last, skip_group_check=True,
                        )
                Gl = cpool.tile([CB, KA], f32)
                nc.vector.tensor_copy(Gl[:], psum_G[:])

            # ============ AllReduce of [G | cnt] + BN stats ============
            with tc.tile_pool(name="pstat", bufs=4, space="PSUM") as ps:
                pcnt = ps.tile([B, 1], f32, tag="st")
                nc.tensor.matmul(pcnt[:], cnt_acc[:], ones_col[:], start=True, stop=True)
                cntl = cpool.tile([B, 1], f32)
                nc.vector.tensor_copy(cntl[:], pcnt[:])

                cc_in = dram.tile([CB + 1, KA], f32)
                cc_out = dram.tile([CB + 1, KA], f32)
                zeros_row = cpool.tile([1, KA], f32)
                nc.vector.memset(zeros_row[:], 0.0)
                nc.sync.dma_start(cc_in[0:CB, :], Gl[:])
                nc.sync.dma_start(cc_in[CB:CB + 1, 0:B], cntl[:])
                nc.sync.dma_start(cc_in[CB:CB + 1, B:KA], zeros_row[:, 0:KA - B])
                nc.gpsimd.collective_compute(
                    "AllReduce", AluOp.add,
                    replica_groups=[list(range(n_cores))],
                    ins=[cc_in.opt()], outs=[cc_out.opt()],
                )
                G_sb = cpool.tile([CB, KA], f32)
                nc.sync.dma_start(G_sb[:], cc_out[0:CB, :])
                cnt_sb = cpool.tile([B, 1], f32)
                nc.sync.dma_start(cnt_sb[:], cc_out[CB:CB + 1, 0:B])

                # ---- small-weights loads ----
                W1_sb = cpool.tile([CB, CO], f32)
                nc.sync.dma_start(W1_sb[:], W_ext[0:CB, :])
                Wt_sb = cpool.tile([CT, CO], f32)
                nc.sync.dma_start(Wt_sb[:], W_ext[CB:CB + CT, :])
                tf_sb = cpool.tile([B, CT], f32)
                nc.sync.dma_start(tf_sb[:], tf_ext[:])
                gam_sb = cpool.tile([CO, 1], f32)
                nc.sync.dma_start(gam_sb[:], gam_ext[:][:, None])
                beta_sb = cpool.tile([CO, 1], f32)
                nc.sync.dma_start(beta_sb[:], beta_ext[:][:, None])

                # T = text_feats @ W[CB:]  (via transpose of text_feats)
                p_tfT = ps.tile([CT, B], f32, tag="st")
                nc.tensor.transpose(p_tfT[:], tf_sb[:], ident[0:B, 0:B])
                tfT_sb = cpool.tile([CT, B], f32)
                nc.vector.tensor_copy(tfT_sb[:], p_tfT[:])
                p_T = ps.tile([B, CO], f32, tag="st")
                nc.tensor.matmul(p_T[:], tfT_sb[:], Wt_sb[:], start=True, stop=True)
                T_sb = cpool.tile([B, CO], f32)
                nc.vector.tensor_copy(T_sb[:], p_T[:])

                # S = (G_FB).T : [B, CB]
                p_S = ps.tile([B, CB], f32, tag="st")
                nc.tensor.transpose(p_S[:], G_sb[:, CB:KA], ident[0:CB, 0:CB])
                S_sb = cpool.tile([B, CB], f32)
                nc.vector.tensor_copy(S_sb[:], p_S[:])

                T2_sb = cpool.tile([B, CO], f32)
                nc.vector.tensor_scalar_mul(T2_sb[:], T_sb[:], 2.0)
                TT2_sb = cpool.tile([B, CO], f32)
                nc.vector.tensor_tensor(TT2_sb[:], T_sb[:], T_sb[:], AluOp.mult)

                # B1 = G_FF @ W1 + S.T @ (2T)
                p_B1 = ps.tile([CB, CO], f32, tag="st")
                nc.tensor.matmul(p_B1[:], G_sb[:, 0:CB], W1_sb[:], start=True, stop=False)
                nc.tensor.matmul(p_B1[:], S_sb[:], T2_sb[:], start=False, stop=True)
                Q_sb = cpool.tile([CB, CO], f32)
                nc.vector.tensor_tensor(Q_sb[:], W1_sb[:], p_B1[:], AluOp.mult)

                # E2 = colsum(Q) + (T*T).T @ cnt   [CO, 1]
                p_E2 = ps.tile([CO, 1], f32, tag="st")
                nc.tensor.matmul(p_E2[:], Q_sb[:], ones_col[0:CB, :], start=True, stop=False)
                nc.tensor.matmul(p_E2[:], TT2_sb[:], cnt_sb[:], start=False, stop=True)

                # mean = (W1.T colsumF + T.T cnt)/N
                colsF = cpool.tile([CB, 1], f32)
                nc.vector.tensor_reduce(
                    colsF[:], G_sb[:, CB:KA], axis=mybir.AxisListType.X, op=AluOp.add
                )
                p_mean = ps.tile([CO, 1], f32, tag="st")
                nc.tensor.matmul(p_mean[:], W1_sb[:], colsF[:], start=True, stop=False)
                nc.tensor.matmul(p_mean[:], T_sb[:], cnt_sb[:], start=False, stop=True)
                mean_sb = cpool.tile([CO, 1], f32)
                nc.vector.tensor_scalar_mul(mean_sb[:], p_mean[:], 1.0 / n_total)

                # var = E2/N - mean^2 ; inv = gamma / sqrt(var + eps)
                e2n = cpool.tile([CO, 1], f32)
                nc.vector.tensor_scalar_mul(e2n[:], p_E2[:], 1.0 / n_total)
                msq = cpool.tile([CO, 1], f32)
                nc.vector.tensor_tensor(msq[:], mean_sb[:], mean_sb[:], AluOp.mult)
                var_sb = cpool.tile([CO, 1], f32)
                nc.vector.tensor_tensor(var_sb[:], e2n[:], msq[:], AluOp.subtract)
                eps_sb = cpool.tile([CO, 1], f32)
                nc.vector.memset(eps_sb[:], BN_EPS)
                std_sb = cpool.tile([CO, 1], f32)
                nc.scalar.activation(std_sb[:], var_sb[:], ActFn.Sqrt, bias=eps_sb[:])
                rstd_sb = cpool.tile([CO, 1], f32)
                nc.vector.reciprocal(rstd_sb[:], std_sb[:])
                inv_sb = cpool.tile([CO, 1], f32)
                nc.vector.tensor_tensor(inv_sb[:], gam_sb[:], rstd_sb[:], AluOp.mult)
                mi_sb = cpool.tile([CO, 1], f32)
                nc.vector.tensor_tensor(mi_sb[:], mean_sb[:], inv_sb[:], AluOp.mult)
                bmi_sb = cpool.tile([CO, 1], f32)
                nc.vector.tensor_tensor(bmi_sb[:], beta_sb[:], mi_sb[:], AluOp.subtract)

                # rows: inv_row = inv.T, bmi_row = bmi.T  [1, CO]
                p_r1 = ps.tile([1, CO], f32, tag="st")
                nc.tensor.transpose(p_r1[:], inv_sb[:], ident[0:CO, 0:CO])
                inv_row = cpool.tile([1, CO], f32)
                nc.vector.tensor_copy(inv_row[:], p_r1[:])
                p_r2 = ps.tile([1, CO], f32, tag="st")
                nc.tensor.transpose(p_r2[:], bmi_sb[:], ident[0:CO, 0:CO])
                bmi_row = cpool.tile([1, CO], f32)
                nc.vector.tensor_copy(bmi_row[:], p_r2[:])

                # W2[0:CB] = W1 * inv (broadcast via rank-1 matmul)
                p_invb = ps.tile([KA, CO], f32, tag="st")
                nc.tensor.matmul(p_invb[:], ones_row[:], inv_row[:], start=True, stop=True)
                W2_sb = cpool.tile([KA, CO], f32)
                nc.vector.tensor_tensor(W2_sb[0:CB, :], W1_sb[:], p_invb[0:CB, :], AluOp.mult)
                # W2[CB:KA] = T * inv + (beta - mean*inv)
                p_w2b = ps.tile([B, CO], f32, tag="st")
                nc.tensor.matmul(p_w2b[:], ones_row[:, 0:B], bmi_row[:], start=True, stop=True)
                t8_sb = cpool.tile([B, CO], f32)
                nc.vector.tensor_tensor(t8_sb[:], T_sb[:], p_invb[0:B, :], AluOp.mult)
                t8b_sb = cpool.tile([B, CO], f32)
                nc.vector.tensor_tensor(t8b_sb[:], t8_sb[:], p_w2b[:], AluOp.add)
                nc.sync.dma_start(W2_sb[CB:KA, :], t8b_sb[:])
                W2_bf = cpool.tile([KA, CO], bf16)
                nc.vector.tensor_copy(W2_bf[:], W2_sb[:])

            # ================= PASS 2: out = relu(A @ W2) =================
            with (
                tc.tile_pool(name="p2t", bufs=2, space="PSUM") as p2t,
                tc.tile_pool(name="p2x", bufs=2, space="PSUM") as p2x,
            ):
                for ch in range(n_chunk):
                    Fb = load_fb(ch)
                    Mr = mask_view(ch)
                    outc = io.tile([P, jr * CO], f32, tag="outchunk")
                    for g in range(jr // GRP):
                        pT = p2t.tile([KA, GRP * P], f32, tag="pT")
                        px = p2x.tile([P, GRP * P], f32, tag="px")
                        AT = atpool.tile([KA, GRP * P], bf16, tag="at")
                        for k in range(GRP):
                            jj = g * GRP + k
                            # Transposes as normal matmuls against the
                            # identity: bf16 inputs stream at full rate into
                            # fp32 PSUM (transpose-mode would force a bf16
                            # PSUM tile and rejects partition offsets).
                            nc.tensor.matmul(
                                pT[0:CB, k * P:(k + 1) * P],
                                Fb[:, jj * CB:(jj + 1) * CB], ident_bf[:],
                                start=True, stop=True,
                            )
                            nc.tensor.matmul(
                                pT[CB:KA, k * P:(k + 1) * P], Mr[:, :, jj],
                                ident_bf[:],
                                start=True, stop=True, tile_position=(0, 96),
                            )
                        # PSUM->SBUF copy split across DVE and ACT for balance
                        nc.vector.tensor_copy(
                            AT[:, 0:GRP * P // 2], pT[:, 0:GRP * P // 2]
                        )
                        nc.scalar.copy(
                            AT[:, GRP * P // 2:], pT[:, GRP * P // 2:]
                        )
                        for k in range(GRP):
                            nc.tensor.matmul(
                                px[:, k * P:k * P + CO],
                                AT[:, k * P:(k + 1) * P], W2_bf[:],
                                start=True, stop=True,
                            )
                        px_view = px[:].rearrange("p (k c) -> p k c", c=P)[:, :, 0:CO]
                        o0 = g * GRP * CO
                        out_view = outc[:, o0:o0 + GRP * CO].rearrange(
                            "p (k c) -> p k c", c=CO
                        )
                        nc.scalar.activation(out_view, px_view, ActFn.Relu)
                    nc.scalar.dma_start(out_v[ch], outc[:])

    nc.compile()
    return nc


_NC_CACHE = {}


def _get_nc():
    key = (N_CHUNK, J, N_TOTAL, NCORES)
    if key not in _NC_CACHE:
        _NC_CACHE[key] = build_nc()
    return _NC_CACHE[key]


def _run(inputs, **spmd_kwargs):
    F = np.ascontiguousarray(np.asarray(inputs["backbone_F"], dtype=np.float32))
    idx = np.ascontiguousarray(np.asarray(inputs["batch_idx"], dtype=np.int32))
    W = np.ascontiguousarray(np.asarray(inputs["W"], dtype=np.float32))
    text = np.ascontiguousarray(np.asarray(inputs["text_feats"], dtype=np.float32))
    gamma = np.ascontiguousarray(np.asarray(inputs["gamma"], dtype=np.float32))
    beta = np.ascontiguousarray(np.asarray(inputs["beta"], dtype=np.float32))

    nc = _get_nc()
    in_maps = []
    for c in range(NCORES):
        Fs = np.zeros((N_PAD, CB), np.float32)
        Fs[:N_SHARD] = F[c * N_SHARD:(c + 1) * N_SHARD]
        ids = np.full((N_PAD,), B, np.int32)  # pad rows get out-of-range id
        ids[:N_SHARD] = idx[c * N_SHARD:(c + 1) * N_SHARD]
        in_maps.append(
            {"F": Fs, "idx": ids, "W": W, "text": text, "gamma": gamma, "beta": beta}
        )
    res = run_bass_kernel_spmd(nc, in_maps, list(range(NCORES)), **spmd_kwargs)
    out = np.concatenate(
        [res.results[c]["out"][:N_SHARD] for c in range(NCORES)], axis=0
    )
    return out, res


def kernel(**inputs) -> np.ndarray:
    return _run(inputs)[0]
